# revision 21
# baseline (speedup 1.0000x reference)
"""Trainium2 Bass kernel for the CLIT-style sparse local attention module.

Strategy (8 NeuronCores, SPMD):
  - core c handles batch b = c // 4 and query chunk qc = c % 4 (1024 queries).
  - Each core recomputes the 5 convs for its batch (3x3 conv as shifted
    matmuls in bf16, fp32 PSUM accumulation).
  - q/k/v conv outputs are PE-transposed to pixel-major rows in DRAM:
    kv_rows [4486, 384] (k|v interleaved per pixel, 3-row zero apron so
    window reads never leave the tensor) and q_rows [4226, 256] (q channels
    + fp32-as-2xbf16 input image channels for the bilinear base term).
  - The 7x7 window gather runs as dma_gather of 7-pixel row segments
    (one descriptor per (query, window-row): 896 descriptors/tile instead
    of 6272); out-of-image positions read garbage neighbors/apron and are
    zeroed by the attention mask, exactly like the reference.
  - Attention (QK logits, softmax) on DVE in fp32 from bf16 operands;
    attention-weighted v is PE-transposed into the [K, Q] layout consumed
    by the 9410x256 first MLP matmul (bf16, fp32 PSUM).
  - Host precomputes gather indices / bilinear weights / masks from
    sample_coord (pure index math) and weight layout shuffles.
"""

import sys

sys.path.insert(0, "/opt/trn_rl_repo")

import numpy as np
import ml_dtypes

# ---------------- problem constants (hardcoded per contract) ----------------
B, CH_IN, H, W = 2, 3, 64, 64
Q = 4096
DIM, HEAD, R = 192, 8, 3
RR = 2 * R + 1
RA = RR * RR          # 49
HD = DIM // HEAD      # 24
ENC = 64
HID = 256
P = H * W             # 4096 pixels
N_CORES = 8
QC = Q * B // N_CORES  # 1024 queries per core
NT = QC // 128         # 8 query tiles per core

KV_ELEM = 2 * DIM                 # 384 (k row + v row, bf16)
KV_PAD = 195                      # 3 rows + 3 px apron before pixel 0
KV_ROWS = P + 2 * KV_PAD          # 4486
Q_PAD = 65                        # 1 row + 1 px apron
Q_ROWS = P + 2 * Q_PAD            # 4226 (max 2-px run start 4224)
DYW = RR * DIM                    # 1344 columns per window-row chunk
DYW_P = 1408                      # padded to 11 x 128
DY_BLOCKS = DYW_P // 128          # 11
KBLK = RR * DY_BLOCKS             # 77 K-blocks for MLP layer 0

f32 = np.float32
bf16 = ml_dtypes.bfloat16

_PROGRAM = None  # cached compiled Bass program


# ============================ device program ================================

def build_program():
    import concourse.bass as bass
    import concourse.tile as tile
    from concourse import bacc, mybir

    dt = mybir.dt

    nc = bacc.Bacc("TRN2", target_bir_lowering=False, debug=False,
                   enable_asserts=False, num_devices=N_CORES)

    def din(name, shape, dtype):
        return nc.dram_tensor(name, list(shape), dtype, kind="ExternalInput").ap()

    # ---- inputs (per-core data) ----
    inp_col = din("inp_col", [27, P], dt.bfloat16)
    inp_hilo = din("inp_hilo", [128, P // 128, 6], dt.bfloat16)
    w_enc = din("w_enc", [27, ENC], dt.bfloat16)
    w_chp = din("w_chp", [128, 3, DIM], dt.bfloat16)
    w_ch2 = din("w_ch2", [ENC, 3, DIM], dt.bfloat16)
    w_qkv0 = din("w_qkv0", [128, 3, 9, DIM], dt.bfloat16)
    w_qkv1p = din("w_qkv1p", [128, 3, 3, DIM], dt.bfloat16)
    w_qkv1k2 = din("w_qkv1k2", [64, 3, 3, DIM], dt.bfloat16)
    qkvb = din("qkvb", [128, 6], dt.float32)
    enc_b = din("enc_b", [ENC, 1], dt.float32)
    ch_b = din("ch_b", [128, 2], dt.float32)
    m0w = din("m0w", [128, KBLK, HID], dt.bfloat16)
    m13w = din("m13w", [128, 6, HID], dt.bfloat16)
    m4w = din("m4w", [128, 2, 3], dt.bfloat16)
    bmlp = din("bmlp", [128, 8], dt.float32)
    b4 = din("b4", [128, 3], dt.float32)
    ident = din("ident", [128, 128], dt.bfloat16)
    kvidx = din("kvidx", [128, NT, RR * 8], dt.int16)
    qidx = din("qidx", [128, NT, 16], dt.int16)
    maskt = din("maskt", [128, NT, RA], dt.float32)
    qwt = din("qwt", [128, NT, 4], dt.bfloat16)
    qwbt = din("qwbt", [128, NT, 4], dt.float32)
    out = nc.dram_tensor("out", [QC, 3], dt.float32, kind="ExternalOutput").ap()

    with tile.TileContext(nc) as tc:
        with tc.tile_pool(name="dram", bufs=1, space="DRAM") as dp:
            kv_rows = dp.tile([KV_ROWS, KV_ELEM], dt.bfloat16)
            q_rows = dp.tile([Q_ROWS, 256], dt.bfloat16)

            _convs(nc, tc, mybir, locals())
            _attention(nc, tc, mybir, locals())

    nc.compile()
    return nc


def _convs(nc, tc, mybir, env):
    dt = mybir.dt
    AF = mybir.ActivationFunctionType

    inp_col, w_enc = env["inp_col"], env["w_enc"]
    w_chp, w_ch2 = env["w_chp"], env["w_ch2"]
    w_qkv0, w_qkv1p, w_qkv1k2 = env["w_qkv0"], env["w_qkv1p"], env["w_qkv1k2"]
    qkvb = env["qkvb"]
    enc_b, ch_b, inp_hilo = env["enc_b"], env["ch_b"], env["inp_hilo"]
    kv_rows, q_rows = env["kv_rows"], env["q_rows"]

    with (
        tc.tile_pool(name="cw", bufs=1) as cw,
        tc.tile_pool(name="cfeat", bufs=1) as cf,
        tc.tile_pool(name="cpsum", bufs=2, space="PSUM") as cp,
        tc.tile_pool(name="qpsum", bufs=2, space="PSUM") as cpq,
        tc.tile_pool(name="ctpsum", bufs=2, space="PSUM") as cpt,
        tc.tile_pool(name="cstage", bufs=3) as cs,
    ):
        col_sb = cw.tile([27, P], dt.bfloat16)
        nc.sync.dma_start(col_sb[:], inp_col)
        wenc_sb = cw.tile([27, ENC], dt.bfloat16)
        nc.sync.dma_start(wenc_sb[:], w_enc)
        wchp_sb = cw.tile([128, 3, DIM], dt.bfloat16)
        nc.sync.dma_start(wchp_sb[:], w_chp)
        wch2_sb = cw.tile([ENC, 3, DIM], dt.bfloat16)
        nc.sync.dma_start(wch2_sb[:], w_ch2)
        wq0 = cw.tile([128, 3, 9, DIM], dt.bfloat16)
        nc.sync.dma_start(wq0[:], w_qkv0)
        wq1p = cw.tile([128, 3, 3, DIM], dt.bfloat16)
        nc.sync.dma_start(wq1p[:], w_qkv1p)
        wq1k2 = cw.tile([64, 3, 3, DIM], dt.bfloat16)
        nc.sync.dma_start(wq1k2[:], w_qkv1k2)
        qkvb_sb = cw.tile([128, 6], dt.float32)
        nc.sync.dma_start(qkvb_sb[:], qkvb)
        encb_sb = cw.tile([ENC, 1], dt.float32)
        nc.sync.dma_start(encb_sb[:], enc_b)
        chb_sb = cw.tile([128, 2], dt.float32)
        nc.sync.dma_start(chb_sb[:], ch_b)
        hilo_sb = cw.tile([128, P // 128, 6], dt.bfloat16)
        nc.sync.dma_start(hilo_sb[:], inp_hilo)
        id_c = cw.tile([128, 128], dt.bfloat16)
        nc.sync.dma_start(id_c[:], env["ident"])

        # zero the DRAM row aprons (kv: 195 rows x 384 = 128x585; q: 65x256)
        zt = cw.tile([128, 585], dt.bfloat16)
        nc.vector.memset(zt[:], 0.0)
        kvf = kv_rows[:, :].flatten()
        nc.sync.dma_start(kvf[0: KV_PAD * KV_ELEM]
                          .rearrange("(p a) -> p a", p=128), zt[:])
        nc.sync.dma_start(kvf[(KV_PAD + P) * KV_ELEM: KV_ROWS * KV_ELEM]
                          .rearrange("(p a) -> p a", p=128), zt[:, 0:585])
        qf = q_rows[:, :].flatten()
        nc.sync.dma_start(qf[0: Q_PAD * 256]
                          .rearrange("(p a) -> p a", p=128), zt[:, 0:130])
        nc.sync.dma_start(qf[(Q_PAD + P) * 256: Q_ROWS * 256]
                          .rearrange("(p a) -> p a", p=128), zt[:, 0:130])

        encp = cf.tile([ENC, 66, 66], dt.bfloat16)
        nc.vector.memset(encp[:], 0.0)
        fp0 = cf.tile([128, 66, 66], dt.bfloat16)
        nc.vector.memset(fp0[:], 0.0)
        fp1 = cf.tile([64, 66, 66], dt.bfloat16)
        nc.vector.memset(fp1[:], 0.0)
        encb2 = cf.tile([128, 66, 66], dt.bfloat16)
        fp1b = cf.tile([128, 66, 66], dt.bfloat16)

        # ---- enc conv ----
        for t in range(8):
            ps = cp.tile([128, 512], dt.float32, tag="cps")
            nc.tensor.matmul(ps[:ENC, :], wenc_sb[:],
                             col_sb[:, t * 512:(t + 1) * 512],
                             start=True, stop=True)
            dst = encp[:, 1 + t * 8: 1 + t * 8 + 8, 1:65]
            nc.scalar.activation(dst, ps[:ENC, :].rearrange("p (a b) -> p a b", a=8),
                                 AF.Identity, bias=encb_sb[:, 0:1])

        # encb2: enc features with a one-column-shifted copy in partitions 64:
        nc.vector.tensor_copy(encb2[0:64, :, :], encp[:])
        nc.vector.tensor_copy(encb2[64:128, :, 0:65], encp[:, :, 1:66])

        # ---- ch conv (kx 0/1 paired into K=128, kx=2 single) ----
        for t in range(8):
            for m, msz in ((0, 128), (1, 64)):
                ps = cp.tile([128, 512], dt.float32, tag="cps")
                for ky in range(3):
                    rhs = encb2[:, t * 8 + ky: t * 8 + ky + 8, 0:64]
                    nc.tensor.matmul(ps[:msz, :],
                                     wchp_sb[:, ky, m * 128: m * 128 + msz],
                                     rhs, start=(ky == 0), stop=False)
                for ky in range(3):
                    rhs = encp[:, t * 8 + ky: t * 8 + ky + 8, 2:66]
                    nc.tensor.matmul(ps[:msz, :],
                                     wch2_sb[:, ky, m * 128: m * 128 + msz],
                                     rhs, start=False, stop=(ky == 2))
                dstp = (fp0 if m == 0 else fp1)
                dst = dstp[:msz, 1 + t * 8: 1 + t * 8 + 8, 1:65]
                nc.scalar.activation(dst,
                                     ps[:msz, :].rearrange("p (a b) -> p a b", a=8),
                                     AF.Identity, bias=chb_sb[:msz, m: m + 1])

        # fp1b: channel-chunk-1 features with one-column-shifted copy
        nc.vector.tensor_copy(fp1b[0:64, :, :], fp1[:])
        nc.vector.tensor_copy(fp1b[64:128, :, 0:65], fp1[:, :, 1:66])

        # ---- q/k/v convs + PE transpose to pixel-major rows ----
        for t in range(8):                    # 512-pixel tiles
            kvstage = cs.tile([128, 4, KV_ELEM], dt.bfloat16, tag="kvstage")
            qstage = cs.tile([128, 4, 256], dt.bfloat16, tag="qstage")
            nc.vector.memset(qstage[:, :, 198:], 0.0)
            nc.vector.tensor_copy(qstage[:, :, 192:198],
                                  hilo_sb[:, t * 4:(t + 1) * 4, :])
            for ci in range(3):               # 0=q, 1=k, 2=v
                for m, msz in ((0, 128), (1, 64)):
                    ps = cpq.tile([128, 512], dt.float32, tag="qkvps")
                    for off in range(9):
                        ky, kx = off // 3, off % 3
                        rhs0 = fp0[:, t * 8 + ky: t * 8 + ky + 8, kx: kx + 64]
                        nc.tensor.matmul(ps[:msz, :],
                                         wq0[:, ci, off, m * 128: m * 128 + msz],
                                         rhs0, start=(off == 0), stop=False)
                    for ky in range(3):
                        rhs1 = fp1b[:, t * 8 + ky: t * 8 + ky + 8, 0:64]
                        nc.tensor.matmul(ps[:msz, :],
                                         wq1p[:, ci, ky, m * 128: m * 128 + msz],
                                         rhs1, start=False, stop=False)
                    for ky in range(3):
                        rhs1 = fp1[:, t * 8 + ky: t * 8 + ky + 8, 2:66]
                        nc.tensor.matmul(ps[:msz, :],
                                         wq1k2[:, ci, ky, m * 128: m * 128 + msz],
                                         rhs1, start=False, stop=(ky == 2))
                    csb = cs.tile([128, 512], dt.bfloat16, tag="convsb")
                    nc.scalar.activation(csb[:msz, :], ps[:msz, :], AF.Identity,
                                         bias=qkvb_sb[:msz, ci * 2 + m: ci * 2 + m + 1])
                    tps = cpt.tile([128, 512], dt.bfloat16, tag="ctps")
                    for blk in range(4):
                        nc.tensor.transpose(
                            tps[:, blk * 128: blk * 128 + msz],
                            csb[:msz, blk * 128:(blk + 1) * 128],
                            id_c[:msz, :msz])
                    coff = (0 if ci < 2 else DIM) + m * 128
                    dstg = qstage if ci == 0 else kvstage
                    nc.scalar.copy(
                        dstg[:, :, coff: coff + msz],
                        tps[:].rearrange("p (a b) -> p a b", a=4)[:, :, 0:msz])
            nc.sync.dma_start(
                kv_rows[KV_PAD + t * 512: KV_PAD + (t + 1) * 512, :]
                .rearrange("(b p) e -> p b e", p=128), kvstage[:])
            nc.sync.dma_start(
                q_rows[Q_PAD + t * 512: Q_PAD + (t + 1) * 512, :]
                .rearrange("(b p) e -> p b e", p=128), qstage[:])


def _attention(nc, tc, mybir, env):
    import concourse.bass as bass
    dt = mybir.dt
    AX = mybir.AxisListType
    AF = mybir.ActivationFunctionType

    kv_rows, q_rows = env["kv_rows"], env["q_rows"]
    m0w, m13w, m4w = env["m0w"], env["m13w"], env["m4w"]
    bmlp, b4, ident = env["bmlp"], env["b4"], env["ident"]
    kvidx, qidx = env["kvidx"], env["qidx"]
    maskt, qwt, qwbt = env["maskt"], env["qwt"], env["qwbt"]
    out = env["out"]

    with (
        tc.tile_pool(name="aw", bufs=1) as aw,
        tc.tile_pool(name="gath", bufs=2) as gp,
        tc.tile_pool(name="attn", bufs=2) as ap_,
        tc.tile_pool(name="attn3", bufs=4) as ap3,
        tc.tile_pool(name="wvp", bufs=4) as wvp,
        tc.tile_pool(name="xtp", bufs=1) as xtp,
        tc.tile_pool(name="hp", bufs=2) as hp,
        tc.tile_pool(name="outp", bufs=1) as op_,
        tc.tile_pool(name="tpsum", bufs=2, space="PSUM") as tp_,
        tc.tile_pool(name="mpsum", bufs=2, space="PSUM") as mp_,
        tc.tile_pool(name="opsum", bufs=2, space="PSUM") as osp,
    ):
        m0w_sb = aw.tile([128, KBLK, HID], dt.bfloat16)
        nc.sync.dma_start(m0w_sb[:], m0w)
        m13_sb = aw.tile([128, 6, HID], dt.bfloat16)
        nc.sync.dma_start(m13_sb[:], m13w)
        m4_sb = aw.tile([128, 2, 3], dt.bfloat16)
        nc.sync.dma_start(m4_sb[:], m4w)
        bm_sb = aw.tile([128, 8], dt.float32)
        nc.sync.dma_start(bm_sb[:], bmlp)
        b4_sb = aw.tile([128, 3], dt.float32)
        nc.sync.dma_start(b4_sb[:], b4)
        id_sb = aw.tile([128, 128], dt.bfloat16)
        nc.sync.dma_start(id_sb[:], ident)
        kvi_sb = aw.tile([128, NT, RR * 8], dt.int16)
        nc.sync.dma_start(kvi_sb[:], kvidx)
        qi_sb = aw.tile([128, NT, 16], dt.int16)
        nc.sync.dma_start(qi_sb[:], qidx)
        mk_sb = aw.tile([128, NT, RA], dt.float32)
        nc.sync.dma_start(mk_sb[:], maskt)
        qw_sb = aw.tile([128, NT, 4], dt.bfloat16)
        nc.sync.dma_start(qw_sb[:], qwt)
        qwb_sb = aw.tile([128, NT, 4], dt.float32)
        nc.sync.dma_start(qwb_sb[:], qwbt)

        out_sb = op_.tile([128, NT, 3], dt.float32)

        h0 = None
        base_tiles = [None, None]

        qv_ap = q_rows[:, :]
        qv_ap = bass.AP(qv_ap.tensor, qv_ap.offset,
                        [[256, Q_ROWS - 1], [1, 512]])
        kv_ap = kv_rows[:, :]
        kv_ap = bass.AP(kv_ap.tensor, kv_ap.offset,
                        [[KV_ELEM, KV_ROWS - 6], [1, RR * KV_ELEM]])

        def issue_gathers(t):
            qg = gp.tile([128, 2, 512], dt.bfloat16, tag="qg")
            nc.gpsimd.dma_gather(qg[:], qv_ap, qi_sb[:, t, :],
                                 num_idxs=256, num_idxs_reg=256, elem_size=512,
                                 elem_step=256, single_packet=False)
            kvg = gp.tile([128, RR, RR * KV_ELEM], dt.bfloat16, tag="kvg")
            nc.gpsimd.dma_gather(kvg[:], kv_ap, kvi_sb[:, t, :],
                                 num_idxs=RR * 128, num_idxs_reg=RR * 128,
                                 elem_size=RR * KV_ELEM, elem_step=KV_ELEM,
                                 single_packet=False)
            return qg, kvg

        pending = issue_gathers(0)
        for t in range(NT):
            qg, kvg = pending
            if t + 1 < NT:
                pending = issue_gathers(t + 1)
            kvg4 = kvg[:].rearrange("p y (x e) -> p y x e", x=RR)
            qg4 = qg[:].rearrange("p y (x e) -> p y x e", x=2)

            # ---------------- q vector + base ----------------
            qprod = ap_.tile([128, 2, 2, DIM], dt.bfloat16, tag="qprod")
            nc.vector.tensor_mul(
                qprod[:], qg4[:, :, :, 0:DIM],
                qw_sb[:, t, :].rearrange("p (a b) -> p a b", a=2)
                .unsqueeze(3).broadcast_to((128, 2, 2, DIM)))
            qf = ap_.tile([128, DIM], dt.float32, tag="qf")
            nc.vector.reduce_sum(qf[:], qprod[:].rearrange("p a b e -> p e a b"),
                                 axis=AX.XY)
            qb = ap_.tile([128, DIM], dt.bfloat16, tag="qb")
            nc.scalar.copy(qb[:], qf[:])

            badd = ap_.tile([128, 2, 2, 3], dt.float32, tag="badd")
            nc.vector.tensor_add(badd[:], qg4[:, :, :, 192:195],
                                 qg4[:, :, :, 195:198])
            bprod = ap_.tile([128, 2, 2, 3], dt.float32, tag="bprod")
            nc.vector.tensor_mul(
                bprod[:], badd[:],
                qwb_sb[:, t, :].rearrange("p (a b) -> p a b", a=2)
                .unsqueeze(3).broadcast_to((128, 2, 2, 3)))
            base_t = ap_.tile([128, 3], dt.float32, tag="base")
            nc.vector.reduce_sum(base_t[:], bprod[:].rearrange("p a b e -> p e a b"),
                                 axis=AX.XY)
            base_tiles[t % 2] = base_t

            # ---------------- attention ----------------
            logits = ap3.tile([128, RA, HEAD], dt.float32, tag="logits")
            for dy in range(RR):
                e = ap3.tile([128, RR, DIM], dt.bfloat16, tag="emul")
                eeng = nc.gpsimd if dy % 2 == 1 else nc.vector
                eeng.tensor_mul(
                    e[:], kvg4[:, dy, :, 0:DIM],
                    qb[:].unsqueeze(1).broadcast_to((128, RR, DIM)))
                nc.vector.reduce_sum(
                    logits[:, dy * RR:(dy + 1) * RR, :],
                    e[:].rearrange("p r (h e) -> p r h e", e=HD), axis=AX.X)

            mask_bc = mk_sb[:, t, :].unsqueeze(2).broadcast_to((128, RA, HEAD))
            nc.vector.tensor_mul(logits[:], logits[:], mask_bc)
            nc.scalar.activation(logits[:], logits[:], AF.Exp)
            ssum = ap_.tile([128, HEAD], dt.float32, tag="ssum")
            nc.vector.reduce_sum(ssum[:], logits[:].rearrange("p r h -> p h r"),
                                 axis=AX.X)
            rec = ap_.tile([128, HEAD], dt.float32, tag="rec")
            nc.vector.reciprocal(rec[:], ssum[:])
            nc.vector.tensor_mul(logits[:], logits[:],
                                 rec[:].unsqueeze(1).broadcast_to((128, RA, HEAD)))
            attnb = ap3.tile([128, RA, HEAD], dt.bfloat16, tag="attnb")
            nc.vector.tensor_mul(attnb[:], logits[:], mask_bc)

            # ---------------- weighted v -> xT (PE transpose) ----------------
            if t % 2 == 0:
                xt_t = xtp.tile([128, KBLK, 256], dt.bfloat16)
            xcol = (t % 2) * 128
            for dy in range(RR):
                wv = wvp.tile([128, DYW_P], dt.bfloat16, tag="wv")
                nc.vector.memset(wv[:, DYW:], 0.0)
                weng = nc.gpsimd if dy % 2 == 0 else nc.vector
                weng.tensor_mul(
                    wv[:, 0:DYW].rearrange("p (r h e) -> p r h e", h=HEAD, e=HD),
                    kvg4[:, dy, :, DIM:2 * DIM].rearrange("p r (h e) -> p r h e",
                                                          e=HD),
                    attnb[:, dy * RR:(dy + 1) * RR, :].unsqueeze(3).broadcast_to(
                        (128, RR, HEAD, HD)))
                for grp, nb in ((0, 4), (1, 4), (2, 3)):
                    tps = tp_.tile([128, 512], dt.bfloat16, tag="tps")
                    for bi_ in range(nb):
                        blk = grp * 4 + bi_
                        nc.tensor.transpose(tps[:, bi_ * 128:(bi_ + 1) * 128],
                                            wv[:, blk * 128:(blk + 1) * 128],
                                            id_sb[:])
                    nc.scalar.copy(
                        xt_t[:, dy * DY_BLOCKS + grp * 4:
                             dy * DY_BLOCKS + grp * 4 + nb, xcol: xcol + 128],
                        tps[:, 0: nb * 128].rearrange("p (a b) -> p a b", a=nb))

            # ---------------- MLP layers 0-3 + head (per 2-tile group) ------
            if t % 2 == 1:
                h0 = hp.tile([128, 2, 256], dt.bfloat16, tag="h")
                for m in range(2):
                    ps = mp_.tile([128, 256], dt.float32, tag="mlp0ps")
                    for kb in range(KBLK):
                        nc.tensor.matmul(ps[:],
                                         m0w_sb[:, kb, m * 128:(m + 1) * 128],
                                         xt_t[:, kb, :],
                                         start=(kb == 0), stop=(kb == KBLK - 1))
                    nc.scalar.activation(h0[:, m, :], ps[:], AF.Relu,
                                         bias=bm_sb[:, m:m + 1])
                cur = h0
                for l in (1, 2, 3):
                    nxt = hp.tile([128, 2, 256], dt.bfloat16, tag="h")
                    for m in range(2):
                        ps = mp_.tile([128, 256], dt.float32, tag="mlp13ps")
                        for kc in range(2):
                            nc.tensor.matmul(
                                ps[:], m13_sb[:, (l - 1) * 2 + kc,
                                              m * 128:(m + 1) * 128],
                                cur[:, kc, :], start=(kc == 0), stop=(kc == 1))
                        nc.scalar.activation(nxt[:, m, :], ps[:], AF.Relu,
                                             bias=bm_sb[:, 2 * l + m: 2 * l + m + 1])
                    cur = nxt
                for tt in range(2):
                    pso = osp.tile([128, 3], dt.float32, tag="pso")
                    for kc in range(2):
                        nc.tensor.matmul(pso[:],
                                         cur[:, kc, tt * 128: tt * 128 + 128],
                                         m4_sb[:, kc, :],
                                         start=(kc == 0), stop=(kc == 1))
                    o1 = ap_.tile([128, 3], dt.float32, tag="o1")
                    nc.vector.tensor_add(o1[:], pso[:], b4_sb[:])
                    nc.vector.tensor_add(out_sb[:, t - 1 + tt, :], o1[:],
                                         base_tiles[tt][:])

        nc.sync.dma_start(
            out.rearrange("(t p) c -> p t c", p=128), out_sb[:])


# ============================ host preparation ==============================

def _host_prep(inputs):
    inp = np.asarray(inputs["inp"], f32)
    sc = np.asarray(inputs["sample_coord"], f32)
    cell = np.asarray(inputs["cell"], f32)

    enc_w = np.asarray(inputs["enc_w"], f32)
    ch_w = np.asarray(inputs["ch_w"], f32)

    w_enc = enc_w.transpose(1, 2, 3, 0).reshape(27, ENC).astype(bf16)
    w_chp = np.zeros((128, 3, DIM), bf16)
    w_ch2 = np.zeros((ENC, 3, DIM), bf16)
    for ky in range(3):
        w_chp[0:64, ky, :] = ch_w[:, :, ky, 0].T.astype(bf16)
        w_chp[64:128, ky, :] = ch_w[:, :, ky, 1].T.astype(bf16)
        w_ch2[:, ky, :] = ch_w[:, :, ky, 2].T.astype(bf16)

    w_qkv0 = np.zeros((128, 3, 9, DIM), bf16)
    w_qkv1p = np.zeros((128, 3, 3, DIM), bf16)
    w_qkv1k2 = np.zeros((64, 3, 3, DIM), bf16)
    qkvb = np.zeros((128, 6), f32)
    for ci, nm in enumerate(("q", "k", "v")):
        wt = np.asarray(inputs[f"{nm}_w"], f32)
        bt = np.asarray(inputs[f"{nm}_b"], f32)
        for off in range(9):
            ky, kx = off // 3, off % 3
            wo = wt[:, :, ky, kx].T
            w_qkv0[:, ci, off, :] = wo[0:128].astype(bf16)
        for ky in range(3):
            w_qkv1p[0:64, ci, ky, :] = wt[:, 128:192, ky, 0].T.astype(bf16)
            w_qkv1p[64:128, ci, ky, :] = wt[:, 128:192, ky, 1].T.astype(bf16)
            w_qkv1k2[:, ci, ky, :] = wt[:, 128:192, ky, 2].T.astype(bf16)
        qkvb[:, ci * 2 + 0] = bt[0:128]
        qkvb[0:64, ci * 2 + 1] = bt[128:192]

    # m0w rows permuted into 7 chunks of 1344 -> 1408 (zero padded)
    m0w_full = np.asarray(inputs["m0w"], f32)
    perm = np.zeros((KBLK * 128, HID), f32)
    for i in range(RR):
        perm[i * DYW_P: i * DYW_P + DYW] = m0w_full[i * DYW: (i + 1) * DYW]
    m0w_dev = np.ascontiguousarray(
        perm.reshape(KBLK, 128, HID).transpose(1, 0, 2)).astype(bf16)

    m13w = np.zeros((128, 6, HID), bf16)
    for l in (1, 2, 3):
        wl = np.asarray(inputs[f"m{l}w"], f32)
        m13w[:, (l - 1) * 2 + 0, :] = wl[0:128].astype(bf16)
        m13w[:, (l - 1) * 2 + 1, :] = wl[128:256].astype(bf16)
    m4w_full = np.asarray(inputs["m4w"], f32)
    m4w = np.stack([m4w_full[0:128], m4w_full[128:256]], 1).astype(bf16)

    b4 = np.broadcast_to(np.asarray(inputs["m4b"], f32)[None, :], (128, 3)).copy()
    enc_bd = np.asarray(inputs["enc_b"], f32).reshape(ENC, 1)
    ch_bd = np.zeros((128, 2), f32)
    ch_bd[:, 0] = np.asarray(inputs["ch_b"], f32)[0:128]
    ch_bd[0:64, 1] = np.asarray(inputs["ch_b"], f32)[128:192]
    ident = np.eye(128, dtype=bf16)

    m0b = np.asarray(inputs["m0b"], f32)
    m0w_tail = m0w_full[RA * DIM: RA * DIM + 2]
    bias_rest = np.zeros((128, 8), f32)
    for l in (1, 2, 3):
        bl = np.asarray(inputs[f"m{l}b"], f32)
        bias_rest[:, 2 * l + 0] = bl[0:128]
        bias_rest[:, 2 * l + 1] = bl[128:256]

    percore = []
    batch_data = []
    for bi in range(B):
        x = inp[bi]
        xp = np.zeros((CH_IN, H + 2, W + 2), f32)
        xp[:, 1:-1, 1:-1] = x
        col = np.zeros((27, P), bf16)
        for c in range(CH_IN):
            for ky in range(3):
                for kx in range(3):
                    col[c * 9 + ky * 3 + kx] = \
                        xp[c, ky: ky + H, kx: kx + W].reshape(-1).astype(bf16)
        # fp32 image as two bf16 halves, pixel-block-major [128, 32, 6]
        xT = x.reshape(3, P).T
        hi = xT.astype(bf16).astype(f32)
        lo = (xT - hi).astype(bf16)
        hilo = np.concatenate([hi.astype(bf16), lo], 1)
        hilo = np.ascontiguousarray(
            hilo.reshape(P // 128, 128, 6).transpose(1, 0, 2))

        rel_cell = cell[bi] * np.array([H, W], f32)
        b0 = m0b + rel_cell @ m0w_tail
        bm = bias_rest.copy()
        bm[:, 0] = b0[0:128]
        bm[:, 1] = b0[128:256]
        batch_data.append((col, hilo, bm))

    sqh = f32(1.0 / np.sqrt(HD))
    d = np.arange(-R, R + 1)
    for core in range(N_CORES):
        bi, qc = core // 4, core % 4
        qs = slice(qc * QC, (qc + 1) * QC)
        cy, cx = sc[bi, qs, 0], sc[bi, qs, 1]
        py = (cy + f32(1.0)) * f32(H * 0.5) - f32(0.5)
        px = (cx + f32(1.0)) * f32(W * 0.5) - f32(0.5)
        iy = np.clip(np.floor(py + f32(0.5)), 0, H - 1).astype(np.int64)
        ix = np.clip(np.floor(px + f32(0.5)), 0, W - 1).astype(np.int64)

        dy, dx = [a.reshape(-1) for a in np.meshgrid(d, d, indexing="ij")]
        yy = iy[:, None] + dy[None, :]
        xx = ix[:, None] + dx[None, :]
        valid = ((yy >= 0) & (yy < H) & (xx >= 0) & (xx < W)).astype(f32)
        # window-row segment start in padded kv_rows: (iy+dy)*64 + ix-3 + pad
        kvstart = ((iy[:, None] + d[None, :]) * W + ix[:, None] - 3 + KV_PAD)

        y0 = np.floor(py)
        x0 = np.floor(px)
        wy, wx = py - y0, px - x0
        y0 = y0.astype(np.int64)
        x0 = x0.astype(np.int64)
        # clamped 2x2 run with corner weights folded onto run slots:
        # border (base) clamps coords; zeros-mode (q) drops invalid corners.
        sy0 = np.clip(y0, 0, H - 2)
        sx0 = np.clip(x0, 0, W - 2)
        wq_eff = np.zeros((QC, 2, 2), f32)
        wb_eff = np.zeros((QC, 2, 2), f32)
        qq = np.arange(QC)
        for ddy, syw in ((0, 1 - wy), (1, wy)):
            for ddx, sxw in ((0, 1 - wx), (1, wx)):
                w = (syw * sxw).astype(f32)
                yc, xc = y0 + ddy, x0 + ddx
                ly = np.clip(yc, 0, H - 1) - sy0
                lx = np.clip(xc, 0, W - 1) - sx0
                wb_eff[qq, ly, lx] += w
                vm = ((yc >= 0) & (yc < H) & (xc >= 0) & (xc < W))
                wq_eff[qq, ly, lx] += w * vm
        cw_ = wb_eff.reshape(QC, 4)
        cval = None
        # bilinear 2-px run start: (sy0+cy)*64 + sx0 + pad
        qstart = ((sy0[:, None] + np.arange(2)[None, :]) * W + sx0[:, None] + Q_PAD)

        kvidx = np.zeros((128, NT, RR * 8), np.int16)
        qidx = np.zeros((128, NT, 16), np.int16)
        maskt = np.zeros((128, NT, RA), f32)
        qwt = np.zeros((128, NT, 4), bf16)
        qwbt = np.zeros((128, NT, 4), f32)
        for t in range(NT):
            ts = slice(t * 128, (t + 1) * 128)
            maskt[:, t, :] = valid[ts]
            qwt[:, t, :] = (wq_eff.reshape(QC, 4)[ts] * sqh).astype(bf16)
            qwbt[:, t, :] = cw_[ts]
            flat = kvstart[ts].T.reshape(-1)          # j = dy*128 + q
            kvidx[:, t, :] = np.tile(flat.reshape(-1, 16).T, (8, 1)).astype(np.int16)
            fq = qstart[ts].T.reshape(-1)             # j = cy*128 + q
            qidx[:, t, :] = np.tile(fq.reshape(-1, 16).T, (8, 1)).astype(np.int16)

        col, hilo, bm = batch_data[bi]
        percore.append({
            "inp_col": col, "inp_hilo": hilo,
            "w_enc": w_enc, "w_chp": w_chp, "w_ch2": w_ch2,
            "w_qkv0": w_qkv0, "w_qkv1p": w_qkv1p, "w_qkv1k2": w_qkv1k2,
            "qkvb": qkvb, "enc_b": enc_bd, "ch_b": ch_bd,
            "m0w": m0w_dev, "m13w": m13w, "m4w": m4w, "bmlp": bm, "b4": b4,
            "ident": ident, "kvidx": kvidx, "qidx": qidx, "maskt": maskt,
            "qwt": qwt, "qwbt": qwbt,
        })
    return percore


# ============================== entry point =================================

def _get_program():
    global _PROGRAM
    if _PROGRAM is None:
        _PROGRAM = build_program()
    return _PROGRAM


def kernel(**inputs):
    from concourse import bass_utils
    nc = _get_program()
    in_maps = _host_prep(inputs)
    res = bass_utils.run_bass_kernel_spmd(nc, in_maps, core_ids=list(range(N_CORES)))
    full = np.empty((B, Q, 3), f32)
    for core in range(N_CORES):
        bi, qc = core // 4, core % 4
        full[bi, qc * QC:(qc + 1) * QC] = res.results[core]["out"]
    return full


if __name__ == "__main__":
    import time
    t0 = time.time()
    nc = _get_program()
    print("built+compiled in", time.time() - t0, "s")
